# revision 67
# baseline (speedup 1.0000x reference)
"""Trainium2 Bass kernel for nn_CPUSelectiveScanMixer (Mamba-style selective scan).

Data-parallel over batch: 8 samples -> 8 NeuronCores, no collectives.

v3 schedule:
 - Conv on PE (diagonal matmuls over a zero-padded xz tile), not DVE stt.
 - Scan phase: ACT does softplus/da/em (em written into a staging tile),
   Pool does bx = b*x (its only big op) + segment memsets, DVE does
   v = em*bx, the hw scan (in-place, the one safe in-place op), yterm,
   reduce tree, y = D*x + ys, gate. No other in-place DVE ops: out tile
   always differs from input regions (in-place tensor ops drop to 1x or
   worse on HW).
 - x_part/y/gate path is bf16 so Pool ops see uniform dtypes.
 - P01 reordered: x + first W_in group transposed first, in_proj starts
   while remaining W_in groups transpose; PE warmup matmuls issued during
   initial DMAs to ramp the PE clock out of its low p-state.
"""
import sys, os

for _p in ("/opt/trn_rl_repo", "/root/.axon_site"):
    if _p not in sys.path and os.path.isdir(_p):
        sys.path.insert(0, _p)

import numpy as np
from contextlib import ExitStack

import concourse.bass as bass
import concourse.bacc as bacc
import concourse.mybir as mybir
from concourse import tile
from concourse import masks
from concourse.bass_utils import run_bass_kernel_spmd

dt = mybir.dt
Alu = mybir.AluOpType
Act = mybir.ActivationFunctionType

S = 1024          # sequence length (per core)
DM = 768          # d_model
DI = 1536         # d_inner
NI = DI // 128    # 12 i-tiles
ND = DM // 128    # 6 d-tiles
NT = S // 128     # 8 t-tiles
NN = 8            # d_state
R = 48            # dt_rank
RBC = R + 2 * NN  # 64
WXM = 104         # padded W_x out rows: dt 0:48, b 64:72, c 96:104
KC = 4            # conv width
B = 8             # batch == n_cores
FS = NN * S       # full scan free size 8192

F32, F16, BF = dt.float32, dt.float16, dt.bfloat16

SIM_SAFE = False  # True: avoid Act.Silu (not implemented in CoreSim)


def _ap3(t, off, dims):
    """3D view of a tile AP: dims is a list of [step, count] free dims."""
    a = t[:]
    return bass.AP(a.tensor, a.offset + off, [a.ap[0]] + dims)


def _silu(nc, sg_p, out_ap, psum_ap, bias, name):
    if SIM_SAFE:
        sg = sg_p.tile([128, 512], F16, tag="sg", name=name)
        nc.scalar.activation(sg[:], psum_ap, Act.Sigmoid,
                             bias=bias if bias is not None else 0.0)
        if bias is not None:
            nc.vector.scalar_tensor_tensor(out_ap, psum_ap, bias, sg[:],
                                           Alu.add, Alu.mult)
        else:
            nc.vector.tensor_mul(out_ap, psum_ap, sg[:])
    else:
        nc.scalar.activation(out_ap, psum_ap, Act.Silu,
                             bias=bias if bias is not None else 0.0)


def build_kernel(nc, tc, ctx):
    # ---------------- DRAM parameters ----------------
    x_d = nc.dram_tensor("x", [S, DM], F32, kind="ExternalInput").ap()
    win_d = nc.dram_tensor("W_in", [2 * DI, DM], F32, kind="ExternalInput").ap()
    cw_d = nc.dram_tensor("conv_w", [DI, KC], F32, kind="ExternalInput").ap()
    cb_d = nc.dram_tensor("conv_b", [DI], F32, kind="ExternalInput").ap()
    wx_d = nc.dram_tensor("W_x", [RBC, DI], F32, kind="ExternalInput").ap()
    wdt_d = nc.dram_tensor("W_dt", [DI, R], F32, kind="ExternalInput").ap()
    bdt_d = nc.dram_tensor("b_dt", [DI], F32, kind="ExternalInput").ap()
    al_d = nc.dram_tensor("A_log", [DI, NN], F32, kind="ExternalInput").ap()
    dsk_d = nc.dram_tensor("D_skip", [DI], F32, kind="ExternalInput").ap()
    wo_d = nc.dram_tensor("W_out", [DM, DI], F32, kind="ExternalInput").ap()
    out_d = nc.dram_tensor("out", [S, DM], F32, kind="ExternalOutput").ap()
    bc_scr = nc.dram_tensor("bc_scratch", [2 * NN, S], BF).ap()

    # ---------------- persistent pools ----------------
    cpool = ctx.enter_context(tc.tile_pool(name="consts", bufs=1))
    iden = cpool.tile([128, 128], F16, tag="iden")
    masks.make_identity(nc, iden[:])
    iden_b = cpool.tile([128, 128], BF, tag="idenb")
    masks.make_identity(nc, iden_b[:])
    iden_f = cpool.tile([128, 128], F32, tag="idenf")
    masks.make_identity(nc, iden_f[:])
    cw = cpool.tile([128, NI * KC], F32, tag="cw")       # conv taps
    cbc = cpool.tile([128, NI], F32, tag="cbc")          # conv bias cols
    bdtc = cpool.tile([128, NI], F32, tag="bdtc")        # dt bias cols
    dskc = cpool.tile([128, NI], F32, tag="dskc")        # D skip cols
    dskb = cpool.tile([128, NI], BF, tag="dskb")         # D skip cols (bf16)
    zcol = cpool.tile([128, 1], BF, tag="zcol")          # zero column
    alf = cpool.tile([128, NI * NN], F32, tag="alf")     # A_log [p,(i,n)]
    anc = cpool.tile([128, NI * NN], F32, tag="anc")     # a = -exp(A_log)
    anb = cpool.tile([128, NI * NN], F32, tag="anb")     # a * 1e-4

    xpart_p = ctx.enter_context(tc.tile_pool(name="xpart", bufs=NI))
    x_part = [xpart_p.tile([128, S], BF, tag="xp", name=f"xp{k}") for k in range(NI)]
    wdtT_p = ctx.enter_context(tc.tile_pool(name="wdtT", bufs=NI))
    W_dtT = [wdtT_p.tile([R, 128], F16, tag="wdtT", name=f"wdtT{k}") for k in range(NI)]
    rep_p = ctx.enter_context(tc.tile_pool(name="rep", bufs=2))
    b_rep = rep_p.tile([128, FS], BF, tag="rep")
    c_rep = rep_p.tile([128, FS], BF, tag="rep")
    dtp_p = ctx.enter_context(tc.tile_pool(name="dtp", bufs=1))
    dt_pT = dtp_p.tile([R, S], F16, tag="dtpT")
    xT_p = ctx.enter_context(tc.tile_pool(name="xT", bufs=ND))
    xT = [xT_p.tile([128, S], F16, tag="xT", name=f"xT{k}") for k in range(ND)]

    # ================ P0+P1: transposes, in_proj(x), conv ================
    with ExitStack() as p01:
        wxT_p = p01.enter_context(tc.tile_pool(name="wxT", bufs=NI))
        W_xT = [wxT_p.tile([128, WXM], BF, tag="wxT", name=f"wxT{k}") for k in range(NI)]
        bct_p = p01.enter_context(tc.tile_pool(name="bct", bufs=2))
        bT = bct_p.tile([NN, S], BF, tag="bct")
        cT = bct_p.tile([NN, S], BF, tag="bct")
        wiT_p = p01.enter_context(tc.tile_pool(name="wiT", bufs=ND))
        W_inT = [wiT_p.tile([128, DI], F16, tag="wiT", name=f"wiT{k}") for k in range(ND)]
        diag_p = p01.enter_context(tc.tile_pool(name="diag", bufs=NI * KC))
        dg = [diag_p.tile([128, 128], F16, tag="dg", name=f"dg{k}")
              for k in range(NI * KC)]

        mm_p = p01.enter_context(tc.tile_pool(name="ps_mm", bufs=2, space="PSUM"))
        cv_p = p01.enter_context(tc.tile_pool(name="ps_cv", bufs=2, space="PSUM"))
        bc_p = p01.enter_context(tc.tile_pool(name="ps_bc", bufs=2, space="PSUM"))
        xz_p = p01.enter_context(tc.tile_pool(name="xz", bufs=3))
        sg_p = p01.enter_context(tc.tile_pool(name="sg", bufs=2))
        wd_p = p01.enter_context(tc.tile_pool(name="wd_st", bufs=4))

        tstack = ExitStack()
        st_p = tstack.enter_context(tc.tile_pool(name="stage", bufs=5))
        ps_p = tstack.enter_context(tc.tile_pool(name="ps_t", bufs=2, space="PSUM"))

        # PE warmup: ramp the p-state clock while initial DMAs are in flight
        wps = ps_p.tile([128, 512], F32, tag="pst", name="warm")
        for w in range(30):
            nc.tensor.matmul(wps[:, 0:128], iden[:], iden[:],
                             start=True, stop=True)

        # x: [S, DM] -> xT[dd] [128d, S] fp16 (cast then fp16 transpose)
        for half in range(2):
            xrow = [st_p.tile([128, DM], F16, tag="xrow", bufs=4,
                              name=f"xrow{half}_{k}") for k in range(4)]
            for q in range(4):
                r = half * 4 + q
                xf = st_p.tile([128, DM], F32, tag="xf32", bufs=2, name=f"xf{half}_{q}")
                nc.sync.dma_start(xf[:], x_d[r * 128:(r + 1) * 128, :])
                nc.scalar.copy(xrow[q][:], xf[:])
            for dd in range(ND):
                pt = ps_p.tile([128, 512], F16, tag="pst")
                for q in range(4):
                    nc.tensor.matmul(pt[:, q * 128:(q + 1) * 128],
                                     xrow[q][:, dd * 128:(dd + 1) * 128],
                                     iden[:], is_transpose=True,
                                     start=True, stop=True)
                nc.vector.tensor_copy(xT[dd][:, half * 512:(half + 1) * 512], pt[:])

        # tiny strided vector loads (emitted after bulk DMAs kick off)
        nc.sync.dma_start(cw[:], bass.AP(cw_d.tensor, 0, [[KC, 128], [128 * KC, NI], [1, KC]]))
        nc.sync.dma_start(cbc[:], bass.AP(cb_d.tensor, 0, [[1, 128], [128, NI]]))
        nc.sync.dma_start(bdtc[:], bass.AP(bdt_d.tensor, 0, [[1, 128], [128, NI]]))
        nc.sync.dma_start(dskc[:], bass.AP(dsk_d.tensor, 0, [[1, 128], [128, NI]]))
        nc.sync.dma_start(alf[:], bass.AP(al_d.tensor, 0, [[NN, 128], [128 * NN, NI], [1, NN]]))
        nc.scalar.activation(anc[:], alf[:], Act.Exp)
        nc.vector.tensor_scalar(anc[:], anc[:], -1.0, None, Alu.mult)
        nc.vector.tensor_scalar(anb[:], anc[:], 1e-4, None, Alu.mult)
        nc.scalar.copy(dskb[:], dskc[:])
        nc.gpsimd.memset(zcol[:], 0.0)
        # conv taps as diagonal PE weight tiles: dg[i*KC+k] = iden * w[i,k]
        for i in range(NI):
            for k in range(KC):
                nc.vector.tensor_scalar(dg[i * KC + k][:], iden[:],
                                        cw[:, i * KC + k:i * KC + k + 1],
                                        None, Alu.mult)

        # W_x: [RBC, DI] -> W_xT[i] [128i, WXM] bf16 (padded col layout)
        wxf = st_p.tile([RBC, DI], F32, tag="wxf", bufs=1)
        nc.sync.dma_start(wxf[:], wx_d[:, :])
        wx_st = st_p.tile([RBC, DI], BF, tag="wxst", bufs=1)
        nc.scalar.copy(wx_st[:], wxf[:])
        for i in range(NI):
            pt = ps_p.tile([128, RBC], BF, tag="pst")
            nc.tensor.matmul(pt[:], wx_st[:, i * 128:(i + 1) * 128],
                             iden_b[0:RBC, 0:RBC],
                             is_transpose=True, start=True, stop=True)
            nc.gpsimd.memset(W_xT[i][:, 48:64], 0.0)
            nc.gpsimd.memset(W_xT[i][:, 72:96], 0.0)
            nc.vector.tensor_copy(W_xT[i][:, 0:48], pt[:, 0:48])
            nc.vector.tensor_copy(W_xT[i][:, 64:72], pt[:, 48:56])
            nc.vector.tensor_copy(W_xT[i][:, 96:104], pt[:, 56:64])

        # W_in x-half group g: rows [4g*128,(4g+4)*128) -> W_inT[dd] cols
        def win_group(g):
            wi_st = [st_p.tile([128, DM], F16, tag="wist", bufs=4,
                               name=f"wist{g}_{k}") for k in range(4)]
            for q in range(4):
                j = g * 4 + q
                wif = st_p.tile([128, DM], F32, tag="wif", bufs=2, name=f"wif{g}_{q}")
                nc.sync.dma_start(wif[:], win_d[j * 128:(j + 1) * 128, :])
                nc.scalar.copy(wi_st[q][:], wif[:])
            for dd in range(ND):
                pt = ps_p.tile([128, 512], F16, tag="pst")
                for q in range(4):
                    nc.tensor.matmul(pt[:, q * 128:(q + 1) * 128],
                                     wi_st[q][:, dd * 128:(dd + 1) * 128],
                                     iden[:], is_transpose=True,
                                     start=True, stop=True)
                nc.vector.tensor_copy(W_inT[dd][:, g * 512:(g + 1) * 512], pt[:])

        win_group(0)

        pbs = [bc_p.tile([WXM, 512], F32, tag="bc", name=f"pb{c}") for c in range(2)]

        # ---- x-half of in_proj + conv(PE diag) + silu + W_x accumulation ----
        for i in range(NI):
            if i == 1:
                win_group(1)
            if i == 5:
                win_group(2)
                tstack.close()
            xz = xz_p.tile([128, S + KC - 1], F16, tag="xz", name=f"xz{i}")
            nc.gpsimd.memset(xz[:, 0:KC - 1], 0.0)
            # dd-outer so each stationary W_inT block loads once for both
            # c-halves (LDWEIGHTS is ~half the PE pipe time in this loop)
            pms = [mm_p.tile([128, 512], F32, tag="mm", name=f"pm{i}_{c}")
                   for c in range(2)]
            for dd in range(ND):
                for c in range(2):
                    nc.tensor.matmul(pms[c][:],
                                     W_inT[dd][:, i * 128:(i + 1) * 128],
                                     xT[dd][:, c * 512:(c + 1) * 512],
                                     start=(dd == 0), stop=(dd == ND - 1))
            for c in range(2):
                nc.vector.tensor_copy(
                    xz[:, KC - 1 + c * 512:KC - 1 + (c + 1) * 512], pms[c][:])
            # causal depthwise conv, k-outer for the same reason
            cvs = [cv_p.tile([128, 512], F32, tag="cv", name=f"cv{i}_{c}")
                   for c in range(2)]
            for k in range(KC):
                for c in range(2):
                    nc.tensor.matmul(cvs[c][:], dg[i * KC + k][:],
                                     xz[:, c * 512 + k:c * 512 + k + 512],
                                     start=(k == 0), stop=(k == KC - 1))
            for c in range(2):
                _silu(nc, sg_p, x_part[i][:, c * 512:(c + 1) * 512], cvs[c][:],
                      cbc[:, i:i + 1], f"sgc{i}_{c}")
            # W_x accumulation (runs as x_part tiles become available)
            for c in range(2):
                nc.tensor.matmul(pbs[c][:], W_xT[i][:],
                                 x_part[i][:, c * 512:(c + 1) * 512],
                                 start=(i == 0), stop=(i == NI - 1))

        # W_dt: [DI, R] -> W_dtT[i] [R, 128i] fp16 (needed only at scan start)
        for i in range(NI):
            wdf = wd_p.tile([128, R], F32, tag="wdf", bufs=2, name=f"wdf{i}")
            nc.sync.dma_start(wdf[:], wdt_d[i * 128:(i + 1) * 128, :])
            wdt_st = wd_p.tile([128, R], F16, tag="wdtst", bufs=2, name=f"wdtst{i}")
            nc.scalar.copy(wdt_st[:], wdf[:])
            pt = cv_p.tile([R, 128], F16, tag="cv", name=f"wdtp{i}")
            nc.tensor.matmul(pt[:], wdt_st[:], iden[:],
                             is_transpose=True, start=True, stop=True)
            nc.vector.tensor_copy(W_dtT[i][:], pt[:])

        # dt_part / b / c extraction + broadcast of b,c across partitions
        for c in range(2):
            c0 = c * 512
            nc.scalar.copy(dt_pT[:, c0:c0 + 512], pbs[c][0:R, :])
            nc.scalar.activation(bT[:, c0:c0 + 512], pbs[c][64:72, :], Act.Tanh)
            nc.scalar.activation(cT[:, c0:c0 + 512], pbs[c][96:104, :], Act.Tanh)
        nc.sync.dma_start(bc_scr[0:NN, :], bT[:])
        nc.sync.dma_start(b_rep[:], bass.AP(bc_scr.tensor, 0, [[0, 128], [1, FS]]))
        nc.gpsimd.dma_start(bc_scr[NN:2 * NN, :], cT[:])
        nc.gpsimd.dma_start(c_rep[:], bass.AP(bc_scr.tensor, FS, [[0, 128], [1, FS]]))

    # ================ P3: selective scan (bf16) + deferred z-half ======
    with ExitStack() as p3:
        da_p = p3.enter_context(tc.tile_pool(name="da", bufs=2))
        em_p = p3.enter_context(tc.tile_pool(name="em", bufs=2))
        bx_p = p3.enter_context(tc.tile_pool(name="bx", bufs=2))
        v_p = p3.enter_context(tc.tile_pool(name="v", bufs=1))
        sp_p = p3.enter_context(tc.tile_pool(name="sp", bufs=1))
        ys_p = p3.enter_context(tc.tile_pool(name="ys", bufs=1))
        y_p = p3.enter_context(tc.tile_pool(name="y", bufs=1))
        sz_p = p3.enter_context(tc.tile_pool(name="siluz", bufs=2))
        wiz_p = p3.enter_context(tc.tile_pool(name="wiz", bufs=2))
        zrow_p = p3.enter_context(tc.tile_pool(name="zrow", bufs=1))
        sgz_p = p3.enter_context(tc.tile_pool(name="sgz", bufs=2))
        dt_ps = p3.enter_context(tc.tile_pool(name="ps_dt", bufs=2, space="PSUM"))
        z_ps = p3.enter_context(tc.tile_pool(name="ps_z", bufs=2, space="PSUM"))
        zt_ps = p3.enter_context(tc.tile_pool(name="ps_zt", bufs=2, space="PSUM"))

        das = [None] * NI
        ems = [None] * NI
        bxs = [None] * NI
        zs = [None] * NI

        def prestage(i):
            # dt path: pd -> softplus -> sp ; da = exp(a*sp) ; em = 1-da
            sp = sp_p.tile([128, S], F16, tag="sp", name=f"sp{i}")
            for c in range(2):
                c0 = c * 512
                pd = dt_ps.tile([128, 512], F32, tag="dt", name=f"pd{i}_{c}")
                nc.tensor.matmul(pd[:], W_dtT[i][:], dt_pT[:, c0:c0 + 512],
                                 start=True, stop=True)
                nc.scalar.activation(sp[:, c0:c0 + 512], pd[:], Act.Exp,
                                     bias=bdtc[:, i:i + 1])
            for c in range(2):
                c0 = c * 512
                nc.scalar.activation(sp[:, c0:c0 + 512],
                                     sp[:, c0:c0 + 512], Act.Ln, bias=1.0)
            da = da_p.tile([128, FS], BF, tag="da", name=f"da{i}")
            for n in range(NN):
                nc.scalar.activation(da[:, n * S:(n + 1) * S], sp[:], Act.Exp,
                                     bias=anb[:, i * NN + n:i * NN + n + 1],
                                     scale=anc[:, i * NN + n:i * NN + n + 1])
            em = em_p.tile([128, FS], BF, tag="em", name=f"em{i}")
            if i == 0:
                # DVE 4x-mode tensor_scalar shortens the scan lead-in;
                # safe now that Pool runs no tensor ops in this phase
                nc.vector.tensor_scalar(em[:], da[:], -1.0, 1.0,
                                        Alu.mult, Alu.add)
            else:
                nc.scalar.activation(em[:], da[:], Act.Copy, bias=1.0, scale=-1.0)
            # bx = x (bcast over n) * b_rep.  DVE: any Pool op co-resident
            # with the DVE stream causes whole-op blocking on the shared
            # SBUF port pair, so Pool gets no tensor work at all here.
            bx = bx_p.tile([128, FS], BF, tag="bx", name=f"bx{i}")
            nc.vector.tensor_tensor(_ap3(bx, 0, [[S, NN], [1, S]]),
                                    _ap3(x_part[i], 0, [[0, NN], [1, S]]),
                                    _ap3(b_rep, 0, [[S, NN], [1, S]]), Alu.mult)
            # zero da at segment starts n>=1 (kills cross-segment chaining)
            nc.gpsimd.memset(da[:, S::S], 0.0)
            das[i], ems[i], bxs[i] = da, em, bx

        def scanstage(i):
            da, em, bx = das[i], ems[i], bxs[i]
            v = v_p.tile([128, FS], BF, tag="v", name=f"v{i}")
            nc.vector.tensor_mul(v[:], em[:], bx[:])
            nc.vector.tensor_tensor_scan(v[:], da[:], v[:], 0.0,
                                         Alu.mult, Alu.add)
            ys = ys_p.tile([128, S], BF, tag="ys", name=f"ys{i}")
            y = y_p.tile([128, S], BF, tag="y", name=f"y{i}")

            def post_ops(t0, t1):
                # yterm into da tile (dead after scan); reduce tree into em
                w = t1 - t0
                nc.vector.tensor_tensor(_ap3(da, t0, [[S, NN], [1, w]]),
                                        _ap3(v, t0, [[S, NN], [1, w]]),
                                        _ap3(c_rep, t0, [[S, NN], [1, w]]),
                                        Alu.mult)
                nc.vector.tensor_tensor(_ap3(em, t0, [[S, 4], [1, w]]),
                                        _ap3(da, t0, [[S, 4], [1, w]]),
                                        _ap3(da, 4 * S + t0, [[S, 4], [1, w]]),
                                        Alu.add)
                nc.vector.tensor_tensor(_ap3(em, 4 * S + t0, [[S, 2], [1, w]]),
                                        _ap3(em, t0, [[S, 2], [1, w]]),
                                        _ap3(em, 2 * S + t0, [[S, 2], [1, w]]),
                                        Alu.add)
                nc.vector.tensor_add(ys[:, t0:t1], em[:, 4 * S + t0:4 * S + t1],
                                     em[:, 5 * S + t0:5 * S + t1])
                nc.vector.scalar_tensor_tensor(y[:, t0:t1],
                                               x_part[i][:, t0:t1],
                                               dskc[:, i:i + 1],
                                               ys[:, t0:t1], Alu.mult, Alu.add)
                nc.vector.tensor_mul(x_part[i][:, t0:t1], y[:, t0:t1],
                                     zs[i][:, t0:t1])

            if i == NI - 1:
                # t-split so out_proj's first row-blocks start earlier
                post_ops(0, 512)
                post_ops(512, S)
            else:
                post_ops(0, S)

        def zstage(i):
            # deferred z-half for this i: silu_z = silu(x @ W_in_z[i])
            sz = sz_p.tile([128, S], BF, tag="sz", name=f"sz{i}")
            zf = zrow_p.tile([128, DM], F32, tag="zf", bufs=1, name=f"zf{i}")
            nc.sync.dma_start(zf[:], win_d[(NI + i) * 128:(NI + i + 1) * 128, :])
            zh = zrow_p.tile([128, DM], F16, tag="zh", bufs=1, name=f"zh{i}")
            nc.scalar.copy(zh[:], zf[:])
            pzt = zt_ps.tile([128, DM], F16, tag="zt")
            for dd in range(ND):
                nc.tensor.matmul(pzt[:, dd * 128:(dd + 1) * 128],
                                 zh[:, dd * 128:(dd + 1) * 128],
                                 iden[:], is_transpose=True,
                                 start=True, stop=True)
            w6 = wiz_p.tile([128, DM], F16, tag="wiz", name=f"wiz{i}")
            nc.scalar.copy(w6[:], pzt[:])
            for c in range(2):
                pz = z_ps.tile([128, 512], F32, tag="z")
                for dd in range(ND):
                    nc.tensor.matmul(pz[:], w6[:, dd * 128:(dd + 1) * 128],
                                     xT[dd][:, c * 512:(c + 1) * 512],
                                     start=(dd == 0), stop=(dd == ND - 1))
                _silu(nc, sgz_p, sz[:, c * 512:(c + 1) * 512], pz[:],
                      None, f"sgz{i}_{c}")
            zs[i] = sz

        for j in range(NI + 1):
            if j < NI:
                prestage(j)
                zstage(j)
            if j >= 1:
                scanstage(j - 1)

    # ================ P4: out_proj ================
    with ExitStack() as p4:
        woT_p = p4.enter_context(tc.tile_pool(name="woT", bufs=NI))
        W_outT = [woT_p.tile([128, DM], BF, tag="woT", name=f"woT{k}") for k in range(NI)]
        pso_p = p4.enter_context(tc.tile_pool(name="ps_to", bufs=2, space="PSUM"))
        outS_p = p4.enter_context(tc.tile_pool(name="outS", bufs=2))
        ps_o = p4.enter_context(tc.tile_pool(name="ps_o", bufs=4, space="PSUM"))

        st4_p = p4.enter_context(tc.tile_pool(name="wo_stage", bufs=1))
        # W_out: [DM, DI] -> W_outT[i] [128i, DM] bf16.  All 6 row DMAs
        # issued upfront (parallel queues); casts pipeline under PE work.
        wofs = [st4_p.tile([128, DI], F32, tag="wof", bufs=ND, name=f"wof{d}")
                for d in range(ND)]
        for d in range(ND):
            nc.sync.dma_start(wofs[d][:], wo_d[d * 128:(d + 1) * 128, :])

        def wo_prep(dd):
            wo_st = st4_p.tile([128, DI], BF, tag="wost4", bufs=3,
                               name=f"wost4_{dd}")
            nc.scalar.copy(wo_st[:], wofs[dd][:])
            for g in range(3):
                pt = pso_p.tile([128, 512], BF, tag="psto")
                for q in range(4):
                    i = g * 4 + q
                    nc.tensor.matmul(pt[:, q * 128:(q + 1) * 128],
                                     wo_st[:, i * 128:(i + 1) * 128],
                                     iden_b[:], is_transpose=True,
                                     start=True, stop=True)
                for q in range(4):
                    i = g * 4 + q
                    nc.vector.tensor_copy(W_outT[i][:, dd * 128:(dd + 1) * 128],
                                          pt[:, q * 128:(q + 1) * 128])

        # all W_out prep first (it hides under the last scan), then r-outer
        # out_proj so early row-blocks stream out as soon as tile 11's first
        # t-half is gated
        for dd in range(ND):
            wo_prep(dd)
        for r in range(NT):
            o = outS_p.tile([128, DM], F32, tag="outS", name=f"o{r}")
            # i-outer so each stationary x_part block loads once per r
            pos = [ps_o.tile([128, 384], F32, tag="po", name=f"po{r}_{h}")
                   for h in range(2)]
            for i in range(NI):
                for half in range(2):
                    nc.tensor.matmul(pos[half][:],
                                     x_part[i][:, r * 128:(r + 1) * 128],
                                     W_outT[i][:, half * 384:(half + 1) * 384],
                                     start=(i == 0), stop=(i == NI - 1))
            for half in range(2):
                nc.vector.tensor_copy(o[:, half * 384:(half + 1) * 384],
                                      pos[half][:])
            nc.sync.dma_start(out_d[r * 128:(r + 1) * 128, :], o[:])


_CACHE = {}


def _get_program():
    if "nc" not in _CACHE:
        nc = bacc.Bacc("TRN2", target_bir_lowering=False, debug=False)
        with tile.TileContext(nc) as tc:
            with ExitStack() as ctx:
                build_kernel(nc, tc, ctx)
        nc.compile()
        _CACHE["nc"] = nc
    return _CACHE["nc"]


def kernel(x, W_in, conv_w, conv_b, W_x, W_dt, b_dt, A_log, D_skip, W_out):
    nc = _get_program()
    x = np.asarray(x, dtype=np.float32)
    shared = {
        "W_in": np.asarray(W_in, np.float32),
        "conv_w": np.asarray(conv_w, np.float32).reshape(DI, KC),
        "conv_b": np.asarray(conv_b, np.float32),
        "W_x": np.asarray(W_x, np.float32),
        "W_dt": np.asarray(W_dt, np.float32),
        "b_dt": np.asarray(b_dt, np.float32),
        "A_log": np.asarray(A_log, np.float32),
        "D_skip": np.asarray(D_skip, np.float32),
        "W_out": np.asarray(W_out, np.float32),
    }
    in_maps = [{"x": np.ascontiguousarray(x[b]), **shared} for b in range(B)]
    res = run_bass_kernel_spmd(nc, in_maps, core_ids=list(range(B)))
    out = np.stack([res.results[b]["out"] for b in range(B)], axis=0)
    return out.astype(np.float32)
